# revision 28
# baseline (speedup 1.0000x reference)
"""Bass/Trainium2 kernel for nn_DiscriminativeCorrelationFilter.

Math
----
Reference computes, per batch b:
  sp = BN(W @ xs_b), tp = BN(W @ xt_b)        (1x1 conv 768->768 + eval-mode BN)
  label from mask centroid (Gaussian)
  f_0 = f_init;  5 iterations:
      r = f_t . tp  (per pixel);  cond = (r*label < 1)
      grad_b = mean(cond * (-label*mask))     (a SCALAR per batch)
      f_{t+1} = (1-LR*LAM) f_t - LR*grad_b*ones
  out_b = f_5 . sp

Because BN(W@x) = inv_std .* (W@x) + cvec (affine per channel) and f_t
stays in span{f_init, ones} (the gradient is a per-batch scalar):
  f_t = a_t * f_init + c_t * ones,  a_t = rho^t  (compile-time)
every channel contraction collapses onto two fixed vectors
    p = W^T (f_init .* inv_std),  q = W^T inv_std          (768 each)
with scalars k1 = f_init.cvec, k2 = sum(cvec):
    f_t . BN(W@x) = a_t (p^T x + k1) + c_t (q^T x + k2)
Device work per batch (features streamed as fp16, ~8 MB/core total):
  target:  psT = [p;q]^T @ xt  (M=2 matmuls), transposed to
           batch-on-partition layout via tiny selection matmuls
  recurrence on ctil_t = c_t/a_t, incremental form (2 DVE ops/iter):
    resp_t = resp_{t-1} + delta_t * (s*lab)
    delta_{t+1} = sum((resp_t < rho^-t) * glm * rho^-(t+1))  [accum_out]
    ctil5 = sum(delta_t)
  search:  bank_h += [p;q]^T @ xs chunks as they stream in (M=2,
           4 chains per PSUM bank via tile_position col-groups) --
           independent of the recurrence, so PE overlaps the DMA
  combine: W3_b = [1; ctil5_b; a5 k1 + a5 k2 ctil5_b] built on-chip
           (diag trick through an identity matmul);
           out_(b,h) = W3_b^T @ [a5 P; a5 Q; ones]  (K=3 matmul)
All weight-derived vectors (p, q, k1, k2, label, glm) are cheap host
precomputes from the small replicated weights (a 768x768 matvec);
the output is exactly f5 . BN(W@xs) re-associated, so the 48 GFLOP of
768x768 projections never run: the kernel is DMA/PE-overlap bound.

Sharding: data-parallel over batch, 4 batches per core on 8 cores.
Engine-op SBUF operands stay at partition bases in {0,32,64,96} (HW
restriction); all other partition rearrangement is done with tiny
selection/identity matmuls on the PE.
"""

import time

import numpy as np
from contextlib import ExitStack

import concourse.bacc as bacc
import concourse.mybir as mybir
import concourse.tile as tile
from concourse.bass_utils import run_bass_kernel_spmd

# ---------------- problem constants (hardcoded; kernel.py must be standalone)
B = 32            # full batch
D = 768           # feature dim
HS = WS = 32      # search spatial
HT = WT = 16      # target spatial
NS = HS * WS      # 1024
NT = HT * WT      # 256
NCORES = 8
BPC = B // NCORES  # 4 batches per core
KC = D // 128      # 6 contraction chunks

LR = 0.1
LAM = 0.01
SIGMA = 2.0
NIT = 5
BN_EPS = 1e-5
RHO = 1.0 - LR * LAM          # 0.999
A5 = RHO ** NIT

F32 = mybir.dt.float32
F16 = mybir.dt.float16   # features stream as fp16 (2-byte, fine mantissa)

_CACHE = {}
XS_DT = F16


def build():
    """Build the per-core Bass program (shapes only; no input values baked)."""
    nc = bacc.Bacc()
    XT_DT = F16
    xt = nc.dram_tensor("xt", (BPC, D, NT), XT_DT, kind="ExternalInput")
    xs = nc.dram_tensor("xs", (BPC, D, NS), XS_DT, kind="ExternalInput")
    cst = nc.dram_tensor("cst", (BPC, 6 * NT + 40), F32, kind="ExternalInput")
    out = nc.dram_tensor("out", (BPC, NS), F32, kind="ExternalOutput")

    AL = mybir.AluOpType
    AF = mybir.ActivationFunctionType

    with tile.TileContext(nc) as tc, ExitStack() as ctx:
        const = ctx.enter_context(tc.tile_pool(name="const", bufs=1))
        feats = ctx.enter_context(tc.tile_pool(name="feats", bufs=1))
        work = ctx.enter_context(tc.tile_pool(name="work", bufs=1))
        psum = ctx.enter_context(tc.tile_pool(name="psum", bufs=8, space="PSUM"))

        # ---- small constant loads
        pqb = nc.dram_tensor("pqb", (D, 2), XS_DT, kind="ExternalInput")
        pqb_sb = const.tile([128, KC, 2], XS_DT, tag="pqb")
        nc.scalar.dma_start(pqb_sb[:, :, :], pqb.rearrange("(k p) c -> p k c", p=128))
        cst_sb = const.tile([BPC, 6 * NT + 40], F32, tag="cst")
        nc.scalar.dma_start(cst_sb[:, :], cst[:, :])
        lab_sb = cst_sb[:, 0:NT]
        glmt_sb = [cst_sb[:, (1 + t) * NT:(2 + t) * NT] for t in range(NIT)]
        karr_sb = cst_sb[:, 6 * NT:6 * NT + 4]
        i4_sb = cst_sb[:, 6 * NT + 4:6 * NT + 8]
        selu_sb = cst_sb[:, 6 * NT + 8:6 * NT + 24]
        sels_sb = cst_sb[:, 6 * NT + 24:6 * NT + 40]

        # ---- feature loads (target first: it gates the serial recurrence)
        xt_sb = []
        for k in range(KC):
            t = feats.tile([128, BPC, NT], XT_DT, tag=f"xt{k}", name=f"xt{k}")
            nc.sync.dma_start(
                t[:, :, :], xt[:, k * 128:(k + 1) * 128, :].rearrange("b p n -> p b n")
            )
            xt_sb.append(t)
        xs_sb = []
        for k in range(KC):
            t = feats.tile([128, BPC, NS], XS_DT, tag=f"xs{k}", name=f"xs{k}")
            sl = xs[:, k * 128:(k + 1) * 128, :]
            if k < KC - 1:
                nc.sync.dma_start(t[:, :, :], sl.rearrange("b p n -> p b n"))
            else:
                # final chunk per-(b,h): each chain's last matmul and combine
                # fire as soon as its own slice lands
                for b in range(BPC):
                    for h in range(2):
                        nc.sync.dma_start(
                            t[:, b, h * 512:(h + 1) * 512],
                            sl[b, :, h * 512:(h + 1) * 512],
                        )
            xs_sb.append(t)

        # ---- target stage: psT[j] (2,512) = [p;q]^T @ xt for batches (2j, 2j+1)
        psT = [psum.tile([2, 512], F32, tag="ps", name=f"psT{j}") for j in range(2)]
        for j in range(2):
            for k in range(KC):
                nc.tensor.matmul(
                    psT[j][:, :],
                    pqb_sb[:, k, :],
                    xt_sb[k][:, 2 * j:2 * j + 2, :],
                    start=(k == 0),
                    stop=(k == KC - 1),
                )

        # ---- move rows to batch-on-partition layout via SBUF->SBUF DMA
        PQs = work.tile([2, 2 * 512], F32, tag="PQs")
        for j in range(2):
            nc.scalar.copy(PQs[:, j * 512:(j + 1) * 512], psT[j][:, :])
        # selection matmuls transpose the psT rows into batch-on-partition
        # PSUM tiles (no SBUF->SBUF DMA latency): psU[b, :] = PQs[0, b-block]
        psU = psum.tile([BPC, NT], F32, tag="ps", name="psU")
        psSv = psum.tile([BPC, NT], F32, tag="ps", name="psSv")
        for b in range(BPC):
            nc.tensor.matmul(
                psU[:, :], selu_sb[0:2, 4 * b:4 * b + 4],
                PQs[0:2, b * NT:(b + 1) * NT],
                start=(b == 0), stop=(b == BPC - 1),
            )
        for b in range(BPC):
            nc.tensor.matmul(
                psSv[:, :], sels_sb[0:2, 4 * b:4 * b + 4],
                PQs[0:2, b * NT:(b + 1) * NT],
                start=(b == 0), stop=(b == BPC - 1),
            )

        # Ulab = (psU + k1) * label ; Slab = (psSv + k2) * label
        Ulab = work.tile([BPC, NT], F32, tag="Ulab")
        Slab = work.tile([BPC, NT], F32, tag="Slab")
        nc.vector.scalar_tensor_tensor(
            Ulab[:, :], psU[:, :], karr_sb[:, 0:1], lab_sb, AL.add, AL.mult
        )
        nc.vector.scalar_tensor_tensor(
            Slab[:, :], psSv[:, :], karr_sb[:, 1:2], lab_sb, AL.add, AL.mult
        )

        # ---- 5-iteration recurrence: resp_t = resp_{t-1} + delta_t*Slab,
        # delta_t = sum(cond_{t-1} * glm * rho^-t) (glm pre-scaled on host)
        resp = work.tile([BPC, NT], F32, tag="resp")
        junk = work.tile([BPC, NT], F32, tag="junk")
        Gt = work.tile([BPC, NIT], F32, tag="Gt")
        nc.vector.scalar_tensor_tensor(
            junk[:, :], Ulab[:, :], 1.0, glmt_sb[0], AL.is_lt, AL.mult,
            accum_out=Gt[:, 0:1],
        )
        for t in range(1, NIT):
            nc.vector.scalar_tensor_tensor(
                resp[:, :], Slab[:, :], Gt[:, t - 1:t],
                Ulab[:, :] if t == 1 else resp[:, :], AL.mult, AL.add
            )
            nc.vector.scalar_tensor_tensor(
                junk[:, :], resp[:, :], float(RHO ** -t), glmt_sb[t],
                AL.is_lt, AL.mult, accum_out=Gt[:, t:t + 1],
            )
        ctil5 = work.tile([BPC, 1], F32, tag="ctil5")
        nc.vector.reduce_sum(ctil5[:, :], Gt[:, :], axis=mybir.AxisListType.X)

        # ---- search stage: [p;q]^T @ xs chunks, 4 chains per PSUM bank
        # (col-group packing: chain (b,h) lives at rows 32b..32b+1 of bank h)
        bank = [psum.tile([128, 512], F32, tag="ps", name=f"bank{h}")
                for h in range(2)]
        for k in range(KC):
            for b in range(BPC):
                for h in range(2):
                    nc.tensor.matmul(
                        bank[h][32 * b:32 * b + 2, :],
                        pqb_sb[:, k, :],
                        xs_sb[k][:, b, h * 512:(h + 1) * 512],
                        tile_position=(0, 32 * b),
                        start=(k == 0),
                        stop=(k == KC - 1),
                    )

        # ---- W3 = per-batch combine weights [1; ctil5; kb] via WL/I4 matmul
        WL = work.tile([BPC, 3], F32, tag="WL")
        nc.vector.memset(WL[:, 0:1], 1.0)
        nc.vector.tensor_scalar(
            out=WL[:, 1:2], in0=ctil5[:, :], scalar1=1.0, scalar2=None, op0=AL.mult,
        )
        nc.vector.tensor_scalar(
            out=WL[:, 2:3], in0=ctil5[:, :], scalar1=karr_sb[:, 3:4],
            scalar2=karr_sb[:, 2:3], op0=AL.mult, op1=AL.add,
        )
        W3ps = psum.tile([3, BPC], F32, tag="ps", name="W3ps")
        nc.tensor.matmul(W3ps[:, :], WL[:, :], i4_sb, start=True, stop=True)
        W3_sb = work.tile([3, BPC], XS_DT, tag="W3_sb")
        nc.vector.tensor_copy(W3_sb[:, :], W3ps[:, :])

        # ---- combine: out_(b,h) = W3_b^T @ [a5*P; a5*Q; ones] then copy out
        PQc = [work.tile([3, 512], XS_DT, tag=f"PQc{i}", name=f"PQc{i}")
               for i in range(2)]
        for i in range(2):
            nc.vector.memset(PQc[i][:, :], 1.0)
        out_row = work.tile([1, BPC, 2, 512], F32, tag="out_row")
        psF = [psum.tile([1, 512], F32, tag="ps", name=f"psF{i}")
               for i in range(2 * BPC)]
        for b in range(BPC):
            for h in range(2):
                j = b * 2 + h
                src_ap = bank[h][32 * b:32 * b + 2, :]
                dst_ap = PQc[j % 2][0:2, :]
                if j % 2 == 0:
                    nc.scalar.activation(dst_ap, src_ap, AF.Copy,
                                         scale=float(A5))
                else:
                    nc.vector.tensor_scalar(
                        out=dst_ap, in0=src_ap, scalar1=float(A5),
                        scalar2=None, op0=AL.mult,
                    )
                nc.tensor.matmul(psF[j][:, :], W3_sb[:, b:b + 1],
                                 PQc[j % 2][:, :], start=True, stop=True)
                if j % 2 == 0:
                    nc.vector.tensor_copy(out_row[0:1, b, h, :], psF[j][0:1, :])
                else:
                    nc.scalar.copy(out_row[0:1, b, h, :], psF[j][0:1, :])
        nc.sync.dma_start(out[0:2, :], out_row[0:1, 0:2, :, :])
        nc.sync.dma_start(out[2:4, :], out_row[0:1, 2:4, :, :])

    nc.finalize()
    return nc


def _host_prep(inputs):
    """Host-side precomputation of p, q, k1, k2, label, glm from small weights."""
    mask = np.asarray(inputs["target_mask"], np.float32).reshape(B, NT)
    W = np.asarray(inputs["conv_w"], np.float64)
    cb = np.asarray(inputs["conv_b"], np.float64)
    gamma = np.asarray(inputs["bn_gamma"], np.float64)
    beta = np.asarray(inputs["bn_beta"], np.float64)
    mean = np.asarray(inputs["bn_mean"], np.float64)
    var = np.asarray(inputs["bn_var"], np.float64)
    f0 = np.asarray(inputs["filter_init"], np.float64).reshape(D)

    inv_std = gamma / np.sqrt(var + BN_EPS)
    cvec = (cb - mean) * inv_std + beta
    p = W.T @ (f0 * inv_std)
    q = W.T @ inv_std
    k1 = float(f0 @ cvec)
    k2 = float(cvec.sum())
    pqh = np.stack([p, q], axis=1).astype(np.float32)          # (768, 2)
    karr_row = np.array([k1, k2, A5 * k1, A5 * k2], np.float64).astype(np.float32)
    karr_h = np.broadcast_to(karr_row, (BPC, 4)).copy()

    # Gaussian label from mask centroid (float32 to mirror the fp32 reference)
    yy, xx = np.meshgrid(
        np.arange(HT, dtype=np.float32), np.arange(WT, dtype=np.float32), indexing="ij"
    )
    yf, xf = yy.reshape(-1), xx.reshape(-1)
    msum = np.maximum(mask.sum(1), np.float32(1.0))
    cy = (mask * yf).sum(1) / msum
    cx = (mask * xf).sum(1) / msum
    d2 = (xf[None, :] - cx[:, None]) ** 2 + (yf[None, :] - cy[:, None]) ** 2
    labh = np.exp(-d2 / np.float32(2.0 * SIGMA * SIGMA)).astype(np.float32)
    glmh = (np.float32(LR / NT) * labh * mask).astype(np.float32)
    glmth = [(glmh * np.float32(RHO ** -(t + 1))).astype(np.float32)
             for t in range(NIT)]
    return pqh, karr_h, labh, glmth


def make_in_maps(inputs):
    sf = np.asarray(inputs["search_features"], np.float32).reshape(B, D, NS)
    sf = sf.astype(np.float16)
    sf = np.ascontiguousarray(sf)
    tf_ = np.asarray(inputs["target_features"], np.float32).reshape(B, D, NT)
    tf_ = tf_.astype(np.float16)
    tf_ = np.ascontiguousarray(tf_)
    pqh, karr_h, labh, glmth = _host_prep(inputs)
    i4h = np.broadcast_to(np.eye(BPC, dtype=np.float32)[None], (NCORES, BPC, BPC))
    # selection matrices, rows 0-1 meaningful: selU[0, 4b+m] = (m == b)
    selu = np.zeros((BPC, 4 * BPC), np.float32)
    sels = np.zeros((BPC, 4 * BPC), np.float32)
    for b in range(BPC):
        selu[0, 4 * b + b] = 1.0
        sels[1, 4 * b + b] = 1.0
    csth = np.concatenate(
        [labh] + glmth +
        [np.broadcast_to(karr_h[None, 0], (B, 4)),
         i4h.reshape(B, BPC),
         np.broadcast_to(selu[None], (NCORES, BPC, 4 * BPC)).reshape(B, -1),
         np.broadcast_to(sels[None], (NCORES, BPC, 4 * BPC)).reshape(B, -1)],
        axis=1,
    ).astype(np.float32)  # (B, 1576)
    in_maps = []
    for c in range(NCORES):
        s = slice(BPC * c, BPC * (c + 1))
        in_maps.append({
            "xt": np.ascontiguousarray(tf_[s]),
            "xs": np.ascontiguousarray(sf[s]),
            "pqb": pqh.astype(np.float16),
            "cst": np.ascontiguousarray(csth[s]),
        })
    return in_maps


def run(inputs, trace=False, **kwargs):
    if "nc" not in _CACHE:
        _CACHE["nc"] = build()
    nc = _CACHE["nc"]
    in_maps = make_in_maps(inputs)
    last_err = None
    for _attempt in range(3):
        try:
            res = run_bass_kernel_spmd(
                nc, in_maps, core_ids=list(range(NCORES)), trace=trace, **kwargs
            )
            break
        except Exception as e:  # transient NRT device faults recover on retry
            last_err = e
            time.sleep(2.0)
    else:
        raise last_err
    outs = [res.results[c]["out"].reshape(BPC, 1, HS, WS) for c in range(NCORES)]
    return np.concatenate(outs, axis=0), res


def kernel(**inputs) -> np.ndarray:
    out, _ = run(inputs)
    return out
